# revision 35
# baseline (speedup 1.0000x reference)
"""Multi-head attention with exclusive post-processing, sharded over 8 trn2 cores.

Sharding: data-parallel over batch (2) x tensor-parallel over heads (16 -> 4/core).
Each core computes a partial transposed output [D, S] (fp16) for its batch from
its 4 heads; the host sums the 4 partials per batch, transposes back, adds bo.

Per-core design (v2 -- pair-fused, ACT-paced):
  Heads are processed in PAIRS sharing the 128-partition dim (even head at
  partitions 0-63, odd head at 64-127):
    QT/KT/VT [128, S]  per pair (feature-major, bf16)
    scores: both heads' score matmuls interleave as concurrent PE row-tiles
      (0,0)/(64,0) into one [128, 2*QC] PSUM tile (h0 cols | h1 cols)
    ONE exp (FD=2*QC) covers the pair -> pT [128, 2*QC] bf16
    attnV per head: lhsT = [V|ones] position-major -> yp [128, QC]
      (rows 0-63 Y, 64-127 softmax denominator)
    exclusive tail runs pair-fused on [128, QC] tiles: ysb relocation copy
      puts the odd head's Y at partitions 64-127; Ln/exp reciprocals
      (exp(-ln x), one ACT table set) and DVE muls cover both heads at the
      same FD cost as one.
    out-projection fuses the pair as a single K=128 contraction
      (Wo rows h0|h1 stacked = sum over both heads for free).
  V' ([V|ones] position-major) accumulates kc-outer DURING the xT input DMA
  stream (16 half-bank PSUM accumulators); inputs split across the sync and
  scalar hardware DMA queues; output is fp16 on the sync queue.
  PSUM: scores 2x[128,1024] (4 banks) + yp 2x[128,512] (2) + filler 2x[128,512]
  (2).  QK tile1 / VT tile1 / out-proj run as fillers inside the ACT-paced
  attention windows via the filler pool.
"""

import os
from contextlib import ExitStack

import ml_dtypes
import numpy as np

import concourse.bass as bass
import concourse.mybir as mybir
import concourse.tile as tile
from concourse import bacc, bass_utils

F32 = mybir.dt.float32
BF16 = mybir.dt.bfloat16
F16 = mybir.dt.float16
AF = mybir.ActivationFunctionType

B, S_FULL, D_FULL, H_FULL = 2, 2048, 1024, 16
HD = 64
N_CORES = 8
HEADS_PER_CORE = H_FULL * B // N_CORES  # 4


def build_nc(S=S_FULL, D=D_FULL, use_bias=False, debug=False):
    P = 128
    HL = HEADS_PER_CORE          # 4 local heads = 2 pairs
    NP = HL // 2                 # pairs (2)
    nH = HL * HD                 # 256
    KC = D // P                  # x contraction chunks (8)
    NKc = S // P                 # key chunks (16)
    QC = 512                     # q chunk (scores pair tile = [128, 2*QC])
    NQ = S // QC                 # 4
    DM = D // P                  # out-proj M tiles (8)

    assert not use_bias, "bias path not implemented (reference biases are zero)"
    _ensure_act_root()
    nc = bacc.Bacc(None, target_bir_lowering=False, num_swdge_queues=2)

    xT_d = nc.dram_tensor("xT", [D, S], BF16, kind="ExternalInput")
    wq_d = nc.dram_tensor("wq", [D, nH], BF16, kind="ExternalInput")
    wk_d = nc.dram_tensor("wk", [D, nH], BF16, kind="ExternalInput")
    wv_d = nc.dram_tensor("wv", [D, nH], BF16, kind="ExternalInput")
    wo_d = nc.dram_tensor("wo", [nH, D], BF16, kind="ExternalInput")
    outT_d = nc.dram_tensor("outT", [D, S], F16, kind="ExternalOutput")
    dbg = {}
    if debug:
        for name, shape, dt in (("dQT0", [128, S], BF16), ("dKT0", [128, S], BF16),
                                ("dQT1", [128, S], BF16), ("dKT1", [128, S], BF16),
                                ("dVT0", [128, S], BF16), ("dVT1", [128, S], BF16),
                                ("dVP", [128, 16 * 4 * 128], BF16),
                                ("dYX0", [128, S], BF16), ("dYX1", [128, S], BF16),
                                ("dYSB", [128, 512], BF16),
                                ("dLND", [128, 512], F32),
                                ("dPT", [128, 1024], BF16)):
            dbg[name] = nc.dram_tensor(name, shape, dt, kind="ExternalOutput")

    with tile.TileContext(nc) as tc, ExitStack() as ctx:
        consts = ctx.enter_context(tc.tile_pool(name="consts", bufs=1))
        psSC = ctx.enter_context(tc.tile_pool(name="psSC", bufs=2, space="PSUM"))
        psYP = ctx.enter_context(tc.tile_pool(name="psYP", bufs=2, space="PSUM"))
        psFL = ctx.enter_context(tc.tile_pool(name="psFL", bufs=2, space="PSUM"))
        pP = ctx.enter_context(tc.tile_pool(name="pP", bufs=4))
        ostgp = ctx.enter_context(tc.tile_pool(name="ostgp", bufs=3))
        stk = ctx.enter_context(tc.tile_pool(name="stk", bufs=2))
        stk2 = ctx.enter_context(tc.tile_pool(name="stk2", bufs=2))
        ysbp = ctx.enter_context(tc.tile_pool(name="ysbp", bufs=3))
        lndp = ctx.enter_context(tc.tile_pool(name="lndp", bufs=3))
        bcp = ctx.enter_context(tc.tile_pool(name="bcp", bufs=3))

        # ---- ACT table preload (single exp+ln set; see _ensure_act_root) ----
        smallc = consts.tile([P, 33], F32, tag="smallc")
        warm = smallc[0:1, 1:33]
        nc.vector.memset(warm, 1.0)
        nc.scalar.activation(out=warm, in_=warm, func=AF.Exp)
        nc.scalar.activation(out=warm, in_=warm, func=AF.Ln)
        epsv = smallc[:, 0:1]
        nc.vector.memset(epsv, 1e-12)

        ones128 = consts.tile([P, HD], BF16, tag="ones128")
        nc.vector.memset(ones128, 1.0)

        # ---- input DMAs split across the three DMA queues ----
        # sync: wq, xT 0/3/6       scalar: wk, wv, xT 2/5      gpsimd: xT 1/4/7, wo
        wq_sb = [consts.tile([P, nH], BF16, tag=f"wq{kc}", name=f"wq{kc}") for kc in range(KC)]
        wk_sb = [consts.tile([P, nH], BF16, tag=f"wk{kc}", name=f"wk{kc}") for kc in range(KC)]
        wv_sb = [consts.tile([P, nH], BF16, tag=f"wv{kc}", name=f"wv{kc}") for kc in range(KC)]
        for kc in range(KC):
            nc.sync.dma_start(out=wq_sb[kc], in_=wq_d.ap()[kc * P:(kc + 1) * P, :])
            nc.scalar.dma_start(out=wk_sb[kc], in_=wk_d.ap()[kc * P:(kc + 1) * P, :])
        for kc in range(KC):
            nc.scalar.dma_start(out=wv_sb[kc],
                                in_=wv_d.ap()[kc * P:(kc + 1) * P, :])
        xT_sb = [consts.tile([P, S], BF16, tag=f"xT{kc}", name=f"xT{kc}") for kc in range(KC)]
        xq = {0: nc.sync, 1: nc.gpsimd, 2: nc.scalar, 3: nc.sync,
              4: nc.gpsimd, 5: nc.scalar, 6: nc.sync, 7: nc.gpsimd}
        for kc in range(KC):
            xq[kc].dma_start(out=xT_sb[kc],
                             in_=xT_d.ap()[kc * P:(kc + 1) * P, :])
        wo_sb = [consts.tile([P, D], BF16, tag=f"wo{p}", name=f"wo{p}") for p in range(NP)]
        for p in range(NP):
            nc.gpsimd.dma_start(out=wo_sb[p], in_=wo_d.ap()[p * P:(p + 1) * P, :])

        QT = [consts.tile([P, S], BF16, tag=f"QT{p}", name=f"QT{p}") for p in range(NP)]
        KT = [consts.tile([P, S], BF16, tag=f"KT{p}", name=f"KT{p}") for p in range(NP)]
        VT = [consts.tile([P, S], BF16, tag=f"VT{p}", name=f"VT{p}") for p in range(NP)]
        vprime = consts.tile([P, NKc, HL, 2 * HD], BF16, tag="vprime")
        nc.vector.memset(vprime[:, :, :, HD:2 * HD], 1.0)

        # ---- PE warm-up burst gated on the first xT chunk: ~4.5us of
        # back-to-back matmuls flips HAM to K=8/8 while the rest of the xT
        # stream lands, so the projection bursts below run at 2.4 GHz ----
        hbt = psFL.tile([HD, P], F32, tag="fl", name="hbwarm")
        for r in range(48):
            nc.tensor.matmul(hbt, lhsT=ones128[0:HD, :],
                             rhs=xT_sb[0][0:HD, 0:P], start=True, stop=True)

        def emit_proj_chunk1024(w_sb, dst, p, q0):
            ps = psSC.tile([P, 2 * QC], F32, tag="sc", name="ps_proj")
            for kc in range(KC):
                for ns in range(0, 2 * QC, 512):
                    nc.tensor.matmul(
                        ps[:, ns:ns + 512],
                        lhsT=w_sb[kc][:, p * P:(p + 1) * P],
                        rhs=xT_sb[kc][:, q0 + ns:q0 + ns + 512],
                        start=(kc == 0), stop=(kc == KC - 1))
            nc.vector.tensor_copy(out=dst[p][:, q0:q0 + 2 * QC], in_=ps)

        # KT tile0 (all keys) + QT tile0 qc0 -- the minimal gate for loop 0;
        # QT0 qc1-3 / VT0 / vprime all run as fillers inside the loops.
        emit_proj_chunk1024(wk_sb, KT, 0, 0)
        emit_proj_chunk1024(wk_sb, KT, 0, 2 * QC)
        qacc = psYP.tile([P, QC], F32, tag="yp", name="qk0acc")
        for kc in range(KC):
            nc.tensor.matmul(
                qacc, lhsT=wq_sb[kc][:, 0:P], rhs=xT_sb[kc][:, 0:QC],
                start=(kc == 0), stop=(kc == KC - 1))
        nc.vector.tensor_copy(out=QT[0][:, 0:QC], in_=qacc)

        # ---- vprime: fully JIT fillers inside loop 0 ----
        def emit_vprime_qt(qt):
            t = psFL.tile([P, QC], F32, tag="fl", name="vacc1")
            acc = t[:, 0:nH]
            for kc in range(KC):
                nc.tensor.matmul(
                    acc, lhsT=xT_sb[kc][:, qt * P:(qt + 1) * P],
                    rhs=wv_sb[kc], start=(kc == 0), stop=(kc == KC - 1))
            nc.vector.tensor_copy(
                out=vprime[:, qt, :, 0:HD],
                in_=acc.rearrange("p (h d) -> p h d", h=HL))

        def emit_proj_chunk(w_sb, dst, p, q0, qw):
            """dst[p][:, q0:q0+qw] = (W pair-slice).T @ xT  (accumulate KC)."""
            ps = psFL.tile([P, qw], F32, tag="fl", name="ps_proj")
            for kc in range(KC):
                for ns in range(0, qw, 512):
                    nc.tensor.matmul(
                        ps[:, ns:ns + 512],
                        lhsT=w_sb[kc][:, p * P:(p + 1) * P],
                        rhs=xT_sb[kc][:, q0 + ns:q0 + ns + 512],
                        start=(kc == 0), stop=(kc == KC - 1))
            nc.vector.tensor_copy(out=dst[p][:, q0:q0 + qw], in_=ps)

        y_excl = [consts.tile([P, S], BF16, tag=f"yx{p}", name=f"yx{p}") for p in range(NP)]

        # ---- D1 pair loop ----
        def emit_d1(p, qc, myfill=()):
            myfill = sorted(myfill, key=lambda x: x[0])
            q0 = qc * QC
            yp0 = psYP.tile([P, QC], F32, tag="yp", name=f"yp0_{p}")
            yp1 = psYP.tile([P, QC], F32, tag="yp", name=f"yp1_{p}")

            def attn_v(pT, kc):
                nc.tensor.matmul(
                    yp0, lhsT=vprime[:, kc, 2 * p, :], rhs=pT[:, 0:QC],
                    start=(kc == 0), stop=(kc == NKc - 1))
                nc.tensor.matmul(
                    yp1, lhsT=vprime[:, kc, 2 * p + 1, :], rhs=pT[:, QC:2 * QC],
                    start=(kc == 0), stop=(kc == NKc - 1))

            prev = None
            for kc in range(NKc):
                sc = psSC.tile([P, 2 * QC], F32, tag="sc", name=f"sc{p}")
                nc.tensor.matmul(
                    sc[:, 0:QC],
                    lhsT=KT[p][0:HD, kc * P:(kc + 1) * P],
                    rhs=QT[p][0:HD, q0:q0 + QC], start=True, stop=True)
                nc.tensor.matmul(
                    sc[:, QC:2 * QC],
                    lhsT=KT[p][HD:P, kc * P:(kc + 1) * P],
                    rhs=QT[p][HD:P, q0:q0 + QC], start=True, stop=True)
                pT = pP.tile([P, 2 * QC], BF16, tag="pt", name=f"pt{p}")
                nc.scalar.activation(out=pT, in_=sc, func=AF.Exp, scale=0.125)
                if prev is not None:
                    attn_v(*prev)
                prev = (pT, kc)
                while myfill and myfill[0][0] <= kc:
                    myfill.pop(0)[1]()
            attn_v(*prev)
            while myfill:
                myfill.pop(0)[1]()

            # extraction: ysb pair (odd head relocated to partitions 64-127),
            # lnden pair
            ysb = ysbp.tile([P, QC], BF16, tag="ysb", name=f"ysb{p}")
            nc.vector.tensor_copy(out=ysb[0:HD, :], in_=yp0[0:HD, :])
            nc.vector.tensor_copy(out=ysb[HD:P, :], in_=yp1[0:HD, :])
            lnden = lndp.tile([P, QC], F32, tag="lnd", name=f"lnden{p}")
            nc.scalar.activation(out=lnden[0:HD, :], in_=yp0[HD:P, :], func=AF.Ln)
            nc.scalar.activation(out=lnden[HD:P, :], in_=yp1[HD:P, :], func=AF.Ln)
            return ysb, lnden, prev[0]

        def heartbeat(dep):
            hb = psFL.tile([HD, HD], F32, tag="fl", name="hb")
            nc.tensor.matmul(hb, lhsT=ones128[0:HD, :], rhs=dep[0:HD, 0:HD],
                             start=True, stop=True)

        # ---- exclusive tail, pair-fused on [128, QC] ----
        def emit_d2(p, qc, ysb, lnden, hb=False):
            q0 = qc * QC
            vth = VT[p][:, q0:q0 + QC]
            # r2 = 1/(sum_hd v^2 + eps) per head, broadcast over 64 partitions
            vsq = stk.tile([P, QC], BF16, tag="vsq")
            nc.vector.tensor_mul(vsq, vth, vth)
            d2B = psFL.tile([P, QC], F32, tag="fl", name="d2B")
            nc.tensor.matmul(d2B[0:HD, :], lhsT=ones128[0:HD, :],
                             rhs=vsq[0:HD, :], start=True, stop=True)
            nc.tensor.matmul(d2B[HD:P, :], lhsT=ones128[HD:P, :],
                             rhs=vsq[HD:P, :], start=True, stop=True)
            lns = bcp.tile([P, QC], F32, tag="lns")
            nc.scalar.activation(out=lns, in_=d2B, func=AF.Ln, bias=epsv)
            r2c = bcp.tile([P, QC], BF16, tag="r2c")
            nc.scalar.activation(out=r2c, in_=lns, func=AF.Exp, scale=-1.0)

            t_yv = stk.tile([P, QC], BF16, tag="t_yv")
            nc.vector.tensor_mul(t_yv, ysb, vth)
            d1B = psFL.tile([P, QC], F32, tag="fl", name="d1B")
            nc.tensor.matmul(d1B[0:HD, :], lhsT=ones128[0:HD, :],
                             rhs=t_yv[0:HD, :], start=True, stop=True)
            nc.tensor.matmul(d1B[HD:P, :], lhsT=ones128[HD:P, :],
                             rhs=t_yv[HD:P, :], start=True, stop=True)

            beta = bcp.tile([P, QC], BF16, tag="bet")
            nc.scalar.activation(out=beta, in_=lnden, func=AF.Exp, scale=-1.0)

            aB = stk2.tile([P, QC], BF16, tag="ab")
            nc.vector.tensor_mul(aB, d1B, r2c)
            if hb:
                heartbeat(aB)
            t2 = stk2.tile([P, QC], BF16, tag="t2")
            nc.vector.tensor_mul(t2, vth, aB)
            u = stk.tile([P, QC], BF16, tag="u")
            nc.vector.tensor_sub(u, ysb, t2)
            if hb:
                heartbeat(u)
            nc.vector.tensor_mul(y_excl[p][:, q0:q0 + QC], u, beta)

        # ---- out-projection for one (mt, qc): K=128 pair-fused ----
        def emit_e_chunk(mt, qc):
            q0 = qc * QC
            ps = psFL.tile([P, QC], F32, tag="fl", name="ps_e")
            for p in range(NP):
                nc.tensor.matmul(
                    ps, lhsT=wo_sb[p][:, mt * P:(mt + 1) * P],
                    rhs=y_excl[p][:, q0:q0 + QC],
                    start=(p == 0), stop=(p == NP - 1))
            ostg = ostgp.tile([P, QC], F16, tag="ostg")
            nc.vector.tensor_copy(out=ostg, in_=ps)
            eng = nc.sync if mt % 2 == 0 else nc.gpsimd
            eng.dma_start(
                out=outT_d.ap()[mt * P:(mt + 1) * P, q0:q0 + QC], in_=ostg)

        # ---- schedule: all pair-0 loops, then pair-1 loops.  Fillers:
        # loop 0: vprime qt2-15 JIT (1/step, 2-step lookahead vs attnV)
        # loop 1: VT0 + start QK1;  loops 2-3: QK1 + VT1
        # loops 5-7: out-proj of qc0/1/2 (1-mt granularity)
        # tail: out-proj qc3.  d2(0,*) lag one loop (VT0 lands in loop 1). ----
        def pc(w_sb, dst, p, q0):
            return lambda: emit_proj_chunk(w_sb, dst, p, q0, QC)

        sched = [[] for _ in range(8)]
        sched[0] = [(s, (lambda qt=s: emit_vprime_qt(qt))) for s in range(NKc)]
        sched[0] += [(13, pc(wq_sb, QT, 0, QC))]
        qk1 = []
        for j in range(4):
            qk1.append(pc(wq_sb, QT, 1, j * QC))
            qk1.append(pc(wk_sb, KT, 1, j * QC))
        vt1 = [pc(wv_sb, VT, 1, j * QC) for j in range(4)]
        # VT0 early in loop 1 (the lagged d2(0,0) filler needs it emitted first)
        sched[1] = [(0, pc(wq_sb, QT, 0, 2 * QC)), (1, pc(wv_sb, VT, 0, 0))]
        sched[1] += [(5 + 2 * j, pc(wv_sb, VT, 0, (j + 1) * QC)) for j in range(3)]
        sched[1] += [(12, pc(wq_sb, QT, 0, 3 * QC))]
        sched[2] = [(3 + 3 * j, qk1[j]) for j in range(5)]
        sched[3] = [(3 + 3 * j, f) for j, f in enumerate(qk1[5:] + vt1[0:2])]
        sched[4] = [(4, vt1[2]), (8, vt1[3])]
        for i, qc in ((5, 0), (6, 1), (7, 2)):
            n_e = 8 if i < 7 else 6
            sched[i] = [(4 + 11 * j // 8, (lambda mt=j, qc=qc: emit_e_chunk(mt, qc)))
                        for j in range(n_e)]

        plan = [(0, qc) for qc in range(NQ)] + [(1, qc) for qc in range(NQ)]
        n_loops = len(plan)
        for i, (p, qc) in enumerate(plan):
            saved = emit_d1(p, qc, sched[i])
            if debug and i == 0:
                nc.sync.dma_start(out=dbg["dYSB"].ap(), in_=saved[0])
                nc.sync.dma_start(out=dbg["dLND"].ap(), in_=saved[1])
                nc.sync.dma_start(out=dbg["dPT"].ap(), in_=saved[2])
            if i < n_loops - 1:
                # spread the exclusive tail's ACT/DVE chain into the next loop
                sched[i + 1].append(
                    (2, (lambda p=p, qc=qc, s=saved: emit_d2(p, qc, s[0], s[1]))))
            else:
                emit_d2(p, qc, *saved[:2], hb=True)
                emit_e_chunk(6, NQ - 2)
                emit_e_chunk(7, NQ - 2)
        for mt in range(DM):
            emit_e_chunk(mt, NQ - 1)
        if debug:
            for nm, t in (("dQT0", QT[0]), ("dKT0", KT[0]), ("dQT1", QT[1]),
                          ("dKT1", KT[1]), ("dVT0", VT[0]), ("dVT1", VT[1]),
                          ("dYX0", y_excl[0]), ("dYX1", y_excl[1])):
                nc.sync.dma_start(out=dbg[nm].ap(), in_=t)
            nc.sync.dma_start(
                out=dbg["dVP"].ap(),
                in_=vprime.rearrange("p a b c -> p (a b c)"))

    nc.finalize()
    return nc


def shard_inputs(x, Wq, bq, Wk, bk, Wv, bv, Wo, bo, n_cores=N_CORES):
    """Full inputs -> per-core input maps (host-side transpose/slice/reshape)."""
    H = Wq.shape[1]
    cores_per_batch = n_cores // x.shape[0]
    hl = H // cores_per_batch
    in_maps = []
    for c in range(n_cores):
        b = c // cores_per_batch
        h0 = (c % cores_per_batch) * hl
        bf = ml_dtypes.bfloat16
        m = {
            "xT": np.ascontiguousarray(x[b].T).astype(bf),
            "wq": np.ascontiguousarray(Wq[:, h0:h0 + hl, :].reshape(Wq.shape[0], -1)).astype(bf),
            "wk": np.ascontiguousarray(Wk[:, h0:h0 + hl, :].reshape(Wk.shape[0], -1)).astype(bf),
            "wv": np.ascontiguousarray(Wv[:, h0:h0 + hl, :].reshape(Wv.shape[0], -1)).astype(bf),
            "wo": np.ascontiguousarray(Wo[h0:h0 + hl].reshape(-1, Wo.shape[2])).astype(bf),
        }
        if _use_bias(bq, bk, bv):
            m["bq"] = np.ascontiguousarray(bq[h0:h0 + hl].reshape(1, -1)).astype(np.float32)
            m["bk"] = np.ascontiguousarray(bk[h0:h0 + hl].reshape(1, -1)).astype(np.float32)
            m["bv"] = np.ascontiguousarray(bv[h0:h0 + hl].reshape(1, -1)).astype(np.float32)
        in_maps.append(m)
    return in_maps


def _use_bias(bq, bk, bv):
    return bool(np.any(bq) or np.any(bk) or np.any(bv))


_ACT_ROOT_READY = False


def _ensure_act_root():
    """Point walrus at an act-table root whose only set is
    natural_log_exp_and_others, so exp and ln share one ACT table set and the
    kernel never pays mid-stream ACT_TABLE_LOADs."""
    global _ACT_ROOT_READY
    if _ACT_ROOT_READY or os.environ.get("BASS_ACT_ROOT_JSON_PATH"):
        _ACT_ROOT_READY = True
        return
    import json
    import tempfile
    from neuronxcc.driver.Job import Job
    from neuronxcc.driver.jobs.support.FindActInfo import findActInfoFile

    orig = findActInfoFile(Job.getPackageDir(), "gen3")
    with open(orig) as f:
        info = json.load(f)
    keep = [e for e in info["act_func_sets"]
            if e["name"] == "natural_log_exp_and_others"]
    if not keep:
        _ACT_ROOT_READY = True
        return
    root = tempfile.mkdtemp(prefix="act_root_")
    src_dir = os.path.dirname(orig)
    for fn in os.listdir(src_dir):
        if fn != "act_info.json":
            os.symlink(os.path.join(src_dir, fn), os.path.join(root, fn))
    info["act_func_sets"] = keep
    with open(os.path.join(root, "act_info.json"), "w") as f:
        json.dump(info, f)
    os.environ["BASS_ACT_ROOT_JSON_PATH"] = os.path.join(root, "act_info.json")

    import concourse.hw_specs as hw_specs
    import concourse.bacc as bacc_mod
    _orig_tables = hw_specs.get_activation_tables

    def _single_set_tables(module_arch):
        tables = _orig_tables(module_arch)
        if "natural_log_exp_and_others" in tables:
            return {"natural_log_exp_and_others": tables["natural_log_exp_and_others"]}
        return tables

    hw_specs.get_activation_tables = _single_set_tables
    bacc_mod.get_activation_tables = _single_set_tables
    _ACT_ROOT_READY = True


_NC_CACHE = {}


def _get_nc(use_bias):
    if use_bias not in _NC_CACHE:
        _NC_CACHE[use_bias] = build_nc(use_bias=use_bias)
    return _NC_CACHE[use_bias]


def run_sharded(inputs, trace=False, trace_cores=None):
    """Run the SPMD kernel; returns (full_output, BassKernelResults)."""
    x, bo = inputs["x"], inputs["bo"]
    use_bias = _use_bias(inputs["bq"], inputs["bk"], inputs["bv"])
    _ensure_act_root()
    nc = _get_nc(use_bias)
    in_maps = shard_inputs(**inputs)
    res = bass_utils.run_bass_kernel_spmd(
        nc, in_maps, core_ids=list(range(N_CORES)),
        trace=trace, trace_cores=trace_cores)
    cores_per_batch = N_CORES // x.shape[0]
    out = np.empty_like(x)
    for b in range(x.shape[0]):
        acc = np.zeros((x.shape[2], x.shape[1]), np.float32)
        for c in range(b * cores_per_batch, (b + 1) * cores_per_batch):
            acc += res.results[c]["outT"].astype(np.float32)
        out[b] = acc.T + bo[None, :]
    return out, res


def kernel(**inputs):
    out, _ = run_sharded(inputs)
    return out


# revision 40
# speedup vs baseline: 1.0301x; 1.0301x over previous
"""Multi-head attention with exclusive post-processing, sharded over 8 trn2 cores.

Sharding: data-parallel over batch (2) x tensor-parallel over heads (16 -> 4/core).
Each core computes a partial transposed output [D, S] (fp16) for its batch from
its 4 heads; the host sums the 4 partials per batch, transposes back, adds bo.

Per-core design (v2 -- pair-fused, ACT-paced):
  Heads are processed in PAIRS sharing the 128-partition dim (even head at
  partitions 0-63, odd head at 64-127):
    QT/KT/VT [128, S]  per pair (feature-major, bf16)
    scores: both heads' score matmuls interleave as concurrent PE row-tiles
      (0,0)/(64,0) into one [128, 2*QC] PSUM tile (h0 cols | h1 cols)
    ONE exp (FD=2*QC) covers the pair -> pT [128, 2*QC] bf16
    attnV per head: lhsT = [V|ones] position-major -> yp [128, QC]
      (rows 0-63 Y, 64-127 softmax denominator)
    exclusive tail runs pair-fused on [128, QC] tiles: ysb relocation copy
      puts the odd head's Y at partitions 64-127; Ln/exp reciprocals
      (exp(-ln x), one ACT table set) and DVE muls cover both heads at the
      same FD cost as one.
    out-projection fuses the pair as a single K=128 contraction
      (Wo rows h0|h1 stacked = sum over both heads for free).
  V' ([V|ones] position-major) accumulates kc-outer DURING the xT input DMA
  stream (16 half-bank PSUM accumulators); inputs split across the sync and
  scalar hardware DMA queues; output is fp16 on the sync queue.
  PSUM: scores 2x[128,1024] (4 banks) + yp 2x[128,512] (2) + filler 2x[128,512]
  (2).  QK tile1 / VT tile1 / out-proj run as fillers inside the ACT-paced
  attention windows via the filler pool.
"""

import os
from contextlib import ExitStack

import ml_dtypes
import numpy as np

import concourse.bass as bass
import concourse.mybir as mybir
import concourse.tile as tile
from concourse import bacc, bass_utils

F32 = mybir.dt.float32
BF16 = mybir.dt.bfloat16
F16 = mybir.dt.float16
AF = mybir.ActivationFunctionType

B, S_FULL, D_FULL, H_FULL = 2, 2048, 1024, 16
HD = 64
N_CORES = 8
HEADS_PER_CORE = H_FULL * B // N_CORES  # 4


def build_nc(S=S_FULL, D=D_FULL, use_bias=False, debug=False):
    P = 128
    HL = HEADS_PER_CORE          # 4 local heads = 2 pairs
    NP = HL // 2                 # pairs (2)
    nH = HL * HD                 # 256
    KC = D // P                  # x contraction chunks (8)
    NKc = S // P                 # key chunks (16)
    QC = 512                     # q chunk (scores pair tile = [128, 2*QC])
    NQ = S // QC                 # 4
    DM = D // P                  # out-proj M tiles (8)

    assert not use_bias, "bias path not implemented (reference biases are zero)"
    _ensure_act_root()
    nc = bacc.Bacc(None, target_bir_lowering=False, num_swdge_queues=2)

    xT_d = nc.dram_tensor("xT", [D, S], BF16, kind="ExternalInput")
    wq_d = nc.dram_tensor("wq", [D, nH], BF16, kind="ExternalInput")
    wk_d = nc.dram_tensor("wk", [D, nH], BF16, kind="ExternalInput")
    wv_d = nc.dram_tensor("wv", [D, nH], BF16, kind="ExternalInput")
    wo_d = nc.dram_tensor("wo", [nH, D], BF16, kind="ExternalInput")
    outT_d = nc.dram_tensor("outT", [D, S], F16, kind="ExternalOutput")
    dbg = {}
    if debug:
        for name, shape, dt in (("dQT0", [128, S], BF16), ("dKT0", [128, S], BF16),
                                ("dQT1", [128, S], BF16), ("dKT1", [128, S], BF16),
                                ("dVT0", [128, S], BF16), ("dVT1", [128, S], BF16),
                                ("dVP", [128, 16 * 4 * 128], BF16),
                                ("dYX0", [128, S], BF16), ("dYX1", [128, S], BF16),
                                ("dYSB", [128, 512], BF16),
                                ("dLND", [128, 512], F32),
                                ("dPT", [128, 1024], BF16)):
            dbg[name] = nc.dram_tensor(name, shape, dt, kind="ExternalOutput")

    with tile.TileContext(nc) as tc, ExitStack() as ctx:
        consts = ctx.enter_context(tc.tile_pool(name="consts", bufs=1))
        psSC = ctx.enter_context(tc.tile_pool(name="psSC", bufs=2, space="PSUM"))
        psYP = ctx.enter_context(tc.tile_pool(name="psYP", bufs=2, space="PSUM"))
        psFL = ctx.enter_context(tc.tile_pool(name="psFL", bufs=2, space="PSUM"))
        pP = ctx.enter_context(tc.tile_pool(name="pP", bufs=4))
        ostgp = ctx.enter_context(tc.tile_pool(name="ostgp", bufs=3))
        stk = ctx.enter_context(tc.tile_pool(name="stk", bufs=2))
        stk2 = ctx.enter_context(tc.tile_pool(name="stk2", bufs=2))
        ysbp = ctx.enter_context(tc.tile_pool(name="ysbp", bufs=3))
        lndp = ctx.enter_context(tc.tile_pool(name="lndp", bufs=3))
        bcp = ctx.enter_context(tc.tile_pool(name="bcp", bufs=3))

        # ---- ACT table preload (single exp+ln set; see _ensure_act_root) ----
        smallc = consts.tile([P, 33], F32, tag="smallc")
        warm = smallc[0:1, 1:33]
        nc.vector.memset(warm, 1.0)
        nc.scalar.activation(out=warm, in_=warm, func=AF.Exp)
        nc.scalar.activation(out=warm, in_=warm, func=AF.Ln)
        epsv = smallc[:, 0:1]
        nc.vector.memset(epsv, 1e-12)

        ones128 = consts.tile([P, HD], BF16, tag="ones128")
        nc.vector.memset(ones128, 1.0)

        # ---- input DMAs split across the three DMA queues ----
        # sync: xT 0/3/6, wq      scalar: wk, xT 2/5, wv      gpsimd: xT 1/4/7, wo
        wq_sb = [consts.tile([P, nH], BF16, tag=f"wq{kc}", name=f"wq{kc}") for kc in range(KC)]
        wk_sb = [consts.tile([P, nH], BF16, tag=f"wk{kc}", name=f"wk{kc}") for kc in range(KC)]
        wv_sb = [consts.tile([P, nH], BF16, tag=f"wv{kc}", name=f"wv{kc}") for kc in range(KC)]
        xT_sb = [consts.tile([P, S], BF16, tag=f"xT{kc}", name=f"xT{kc}") for kc in range(KC)]
        for kc in range(KC):
            nc.sync.dma_start(out=wq_sb[kc], in_=wq_d.ap()[kc * P:(kc + 1) * P, :])
            nc.scalar.dma_start(out=wk_sb[kc], in_=wk_d.ap()[kc * P:(kc + 1) * P, :])
        for kc in range(KC):
            nc.scalar.dma_start(out=wv_sb[kc],
                                in_=wv_d.ap()[kc * P:(kc + 1) * P, :])
        xq = {0: nc.sync, 1: nc.gpsimd, 2: nc.scalar, 3: nc.sync,
              4: nc.gpsimd, 5: nc.scalar, 6: nc.sync, 7: nc.gpsimd}
        for kc in range(KC):
            xq[kc].dma_start(out=xT_sb[kc],
                             in_=xT_d.ap()[kc * P:(kc + 1) * P, :])
        wo_sb = [consts.tile([P, D], BF16, tag=f"wo{p}", name=f"wo{p}") for p in range(NP)]
        for p in range(NP):
            nc.gpsimd.dma_start(out=wo_sb[p], in_=wo_d.ap()[p * P:(p + 1) * P, :])

        QT = [consts.tile([P, S], BF16, tag=f"QT{p}", name=f"QT{p}") for p in range(NP)]
        KT = [consts.tile([P, S], BF16, tag=f"KT{p}", name=f"KT{p}") for p in range(NP)]
        VT = [consts.tile([P, S], BF16, tag=f"VT{p}", name=f"VT{p}") for p in range(NP)]
        vprime = consts.tile([P, NKc, HL, 2 * HD], BF16, tag="vprime")
        nc.vector.memset(vprime[:, :, :, HD:2 * HD], 1.0)

        # ---- PE warm-up burst gated on the first xT chunk: ~4.5us of
        # back-to-back matmuls flips HAM to K=8/8 while the rest of the xT
        # stream lands, so the projection bursts below run at 2.4 GHz ----
        hbt = psFL.tile([HD, P], F32, tag="fl", name="hbwarm")
        for r in range(48):
            nc.tensor.matmul(hbt, lhsT=ones128[0:HD, :],
                             rhs=xT_sb[0][0:HD, 0:P], start=True, stop=True)

        # KT tile0 (all keys) + QT tile0 qc0 -- the minimal gate for loop 0 --
        # accumulate kc-outer DURING the xT stream, in expected arrival order
        # (gpsimd chunks land first, scalar last), so PE never idles >3.4us
        # and HAM stays warm after the burst.  QT0 qc1-3 / VT0 / vprime run
        # as fillers inside the loops.
        arrival = [1, 0, 4, 2, 3, 7, 5, 6]
        kacc = []
        for _ in range(2):
            t = psSC.tile([P, 2 * QC], F32, tag="sc", name="qk0acc")
            kacc.append(t[:, 0:QC])
            kacc.append(t[:, QC:2 * QC])
        qacc = psYP.tile([P, QC], F32, tag="yp", name="qk0acc")
        for n, kc in enumerate(arrival):
            for qc in range(4):
                nc.tensor.matmul(
                    kacc[qc], lhsT=wk_sb[kc][:, 0:P],
                    rhs=xT_sb[kc][:, qc * QC:(qc + 1) * QC],
                    start=(n == 0), stop=(n == KC - 1))
        # QT0 qc0 after the stream (wq rides behind xT on the sync queue)
        for n, kc in enumerate(arrival):
            nc.tensor.matmul(
                qacc, lhsT=wq_sb[kc][:, 0:P], rhs=xT_sb[kc][:, 0:QC],
                start=(n == 0), stop=(n == KC - 1))
        for qc in range(4):
            nc.vector.tensor_copy(out=KT[0][:, qc * QC:(qc + 1) * QC], in_=kacc[qc])
        nc.vector.tensor_copy(out=QT[0][:, 0:QC], in_=qacc)

        # ---- vprime: fully JIT fillers inside loop 0 ----
        def emit_vprime_qt(qt):
            t = psFL.tile([P, QC], F32, tag="fl", name="vacc1")
            acc = t[:, 0:nH]
            for kc in range(KC):
                nc.tensor.matmul(
                    acc, lhsT=xT_sb[kc][:, qt * P:(qt + 1) * P],
                    rhs=wv_sb[kc], start=(kc == 0), stop=(kc == KC - 1))
            nc.vector.tensor_copy(
                out=vprime[:, qt, :, 0:HD],
                in_=acc.rearrange("p (h d) -> p h d", h=HL))

        def emit_proj_chunk(w_sb, dst, p, q0, qw):
            """dst[p][:, q0:q0+qw] = (W pair-slice).T @ xT  (accumulate KC)."""
            ps = psFL.tile([P, qw], F32, tag="fl", name="ps_proj")
            for kc in range(KC):
                for ns in range(0, qw, 512):
                    nc.tensor.matmul(
                        ps[:, ns:ns + 512],
                        lhsT=w_sb[kc][:, p * P:(p + 1) * P],
                        rhs=xT_sb[kc][:, q0 + ns:q0 + ns + 512],
                        start=(kc == 0), stop=(kc == KC - 1))
            nc.vector.tensor_copy(out=dst[p][:, q0:q0 + qw], in_=ps)

        y_excl = [consts.tile([P, S], BF16, tag=f"yx{p}", name=f"yx{p}") for p in range(NP)]

        # ---- D1 pair loop ----
        def emit_d1(p, qc, myfill=()):
            myfill = sorted(myfill, key=lambda x: x[0])
            q0 = qc * QC
            yp0 = psYP.tile([P, QC], F32, tag="yp", name=f"yp0_{p}")
            yp1 = psYP.tile([P, QC], F32, tag="yp", name=f"yp1_{p}")

            def attn_v(pT, kc):
                nc.tensor.matmul(
                    yp0, lhsT=vprime[:, kc, 2 * p, :], rhs=pT[:, 0:QC],
                    start=(kc == 0), stop=(kc == NKc - 1))
                nc.tensor.matmul(
                    yp1, lhsT=vprime[:, kc, 2 * p + 1, :], rhs=pT[:, QC:2 * QC],
                    start=(kc == 0), stop=(kc == NKc - 1))

            prev = None
            for kc in range(NKc):
                sc = psSC.tile([P, 2 * QC], F32, tag="sc", name=f"sc{p}")
                nc.tensor.matmul(
                    sc[:, 0:QC],
                    lhsT=KT[p][0:HD, kc * P:(kc + 1) * P],
                    rhs=QT[p][0:HD, q0:q0 + QC], start=True, stop=True)
                nc.tensor.matmul(
                    sc[:, QC:2 * QC],
                    lhsT=KT[p][HD:P, kc * P:(kc + 1) * P],
                    rhs=QT[p][HD:P, q0:q0 + QC], start=True, stop=True)
                pT = pP.tile([P, 2 * QC], BF16, tag="pt", name=f"pt{p}")
                nc.scalar.activation(out=pT, in_=sc, func=AF.Exp, scale=0.125)
                if prev is not None:
                    attn_v(*prev)
                prev = (pT, kc)
                while myfill and myfill[0][0] <= kc:
                    myfill.pop(0)[1]()
            attn_v(*prev)
            while myfill:
                myfill.pop(0)[1]()

            # extraction: ysb pair (odd head relocated to partitions 64-127),
            # lnden pair
            ysb = ysbp.tile([P, QC], BF16, tag="ysb", name=f"ysb{p}")
            nc.vector.tensor_copy(out=ysb[0:HD, :], in_=yp0[0:HD, :])
            nc.vector.tensor_copy(out=ysb[HD:P, :], in_=yp1[0:HD, :])
            lnden = lndp.tile([P, QC], F32, tag="lnd", name=f"lnden{p}")
            nc.scalar.activation(out=lnden[0:HD, :], in_=yp0[HD:P, :], func=AF.Ln)
            nc.scalar.activation(out=lnden[HD:P, :], in_=yp1[HD:P, :], func=AF.Ln)
            return ysb, lnden, prev[0]

        def heartbeat(dep):
            hb = psFL.tile([HD, HD], F32, tag="fl", name="hb")
            nc.tensor.matmul(hb, lhsT=ones128[0:HD, :], rhs=dep[0:HD, 0:HD],
                             start=True, stop=True)

        # ---- exclusive tail, pair-fused on [128, QC] ----
        def emit_d2(p, qc, ysb, lnden, hb=False):
            q0 = qc * QC
            vth = VT[p][:, q0:q0 + QC]
            # r2 = 1/(sum_hd v^2 + eps) per head, broadcast over 64 partitions
            vsq = stk.tile([P, QC], BF16, tag="vsq")
            nc.vector.tensor_mul(vsq, vth, vth)
            d2B = psFL.tile([P, QC], F32, tag="fl", name="d2B")
            nc.tensor.matmul(d2B[0:HD, :], lhsT=ones128[0:HD, :],
                             rhs=vsq[0:HD, :], start=True, stop=True)
            nc.tensor.matmul(d2B[HD:P, :], lhsT=ones128[HD:P, :],
                             rhs=vsq[HD:P, :], start=True, stop=True)
            lns = bcp.tile([P, QC], F32, tag="lns")
            nc.scalar.activation(out=lns, in_=d2B, func=AF.Ln, bias=epsv)
            r2c = bcp.tile([P, QC], BF16, tag="r2c")
            nc.scalar.activation(out=r2c, in_=lns, func=AF.Exp, scale=-1.0)

            t_yv = stk.tile([P, QC], BF16, tag="t_yv")
            nc.vector.tensor_mul(t_yv, ysb, vth)
            d1B = psFL.tile([P, QC], F32, tag="fl", name="d1B")
            nc.tensor.matmul(d1B[0:HD, :], lhsT=ones128[0:HD, :],
                             rhs=t_yv[0:HD, :], start=True, stop=True)
            nc.tensor.matmul(d1B[HD:P, :], lhsT=ones128[HD:P, :],
                             rhs=t_yv[HD:P, :], start=True, stop=True)

            beta = bcp.tile([P, QC], BF16, tag="bet")
            nc.scalar.activation(out=beta, in_=lnden, func=AF.Exp, scale=-1.0)

            aB = stk2.tile([P, QC], BF16, tag="ab")
            nc.vector.tensor_mul(aB, d1B, r2c)
            if hb:
                heartbeat(aB)
            t2 = stk2.tile([P, QC], BF16, tag="t2")
            nc.vector.tensor_mul(t2, vth, aB)
            u = stk.tile([P, QC], BF16, tag="u")
            nc.vector.tensor_sub(u, ysb, t2)
            if hb:
                heartbeat(u)
            nc.vector.tensor_mul(y_excl[p][:, q0:q0 + QC], u, beta)

        # ---- out-projection for one (mt, qc): K=128 pair-fused ----
        def emit_e_chunk(mt, qc):
            q0 = qc * QC
            ps = psFL.tile([P, QC], F32, tag="fl", name="ps_e")
            for p in range(NP):
                nc.tensor.matmul(
                    ps, lhsT=wo_sb[p][:, mt * P:(mt + 1) * P],
                    rhs=y_excl[p][:, q0:q0 + QC],
                    start=(p == 0), stop=(p == NP - 1))
            ostg = ostgp.tile([P, QC], F16, tag="ostg")
            nc.vector.tensor_copy(out=ostg, in_=ps)
            eng = nc.sync if mt % 2 == 0 else nc.gpsimd
            eng.dma_start(
                out=outT_d.ap()[mt * P:(mt + 1) * P, q0:q0 + QC], in_=ostg)

        # ---- schedule: all pair-0 loops, then pair-1 loops.  Fillers:
        # loop 0: vprime qt2-15 JIT (1/step, 2-step lookahead vs attnV)
        # loop 1: VT0 + start QK1;  loops 2-3: QK1 + VT1
        # loops 5-7: out-proj of qc0/1/2 (1-mt granularity)
        # tail: out-proj qc3.  d2(0,*) lag one loop (VT0 lands in loop 1). ----
        def pc(w_sb, dst, p, q0):
            return lambda: emit_proj_chunk(w_sb, dst, p, q0, QC)

        sched = [[] for _ in range(8)]
        sched[0] = [(s, (lambda qt=s: emit_vprime_qt(qt))) for s in range(NKc)]
        sched[0] += [(13, pc(wq_sb, QT, 0, QC))]
        qk1 = []
        for j in range(4):
            qk1.append(pc(wq_sb, QT, 1, j * QC))
            qk1.append(pc(wk_sb, KT, 1, j * QC))
        vt1 = [pc(wv_sb, VT, 1, j * QC) for j in range(4)]
        sched[1] = [(0, pc(wq_sb, QT, 0, 2 * QC)), (3, pc(wq_sb, QT, 0, 3 * QC))]
        sched[1] += [(5 + 3 * j, pc(wv_sb, VT, 0, j * QC)) for j in range(4)]
        sched[2] = [(1 + 3 * j, qk1[j]) for j in range(5)]
        sched[3] = [(1 + 3 * j, f) for j, f in enumerate(qk1[5:] + vt1[0:2])]
        sched[4] = [(2, vt1[2]), (6, vt1[3])]
        for i, qc in ((5, 0), (6, 1), (7, 2)):
            n_e = 8 if i < 7 else 6
            sched[i] = [(1 + 2 * j, (lambda mt=j, qc=qc: emit_e_chunk(mt, qc)))
                        for j in range(n_e)]

        plan = [(0, qc) for qc in range(NQ)] + [(1, qc) for qc in range(NQ)]
        n_loops = len(plan)
        pending = None
        for i, (p, qc) in enumerate(plan):
            saved = emit_d1(p, qc, sched[i])
            if debug and i == 0:
                nc.sync.dma_start(out=dbg["dYSB"].ap(), in_=saved[0])
                nc.sync.dma_start(out=dbg["dLND"].ap(), in_=saved[1])
                nc.sync.dma_start(out=dbg["dPT"].ap(), in_=saved[2])
            if pending is not None:
                emit_d2(*pending)
                pending = None
            if p == 0:
                pending = (p, qc, saved[0], saved[1])
            else:
                emit_d2(p, qc, *saved[:2], hb=(i == n_loops - 1))
                if i == n_loops - 1:
                    emit_e_chunk(6, NQ - 2)
                    emit_e_chunk(7, NQ - 2)
        for mt in range(DM):
            emit_e_chunk(mt, NQ - 1)
        if debug:
            for nm, t in (("dQT0", QT[0]), ("dKT0", KT[0]), ("dQT1", QT[1]),
                          ("dKT1", KT[1]), ("dVT0", VT[0]), ("dVT1", VT[1]),
                          ("dYX0", y_excl[0]), ("dYX1", y_excl[1])):
                nc.sync.dma_start(out=dbg[nm].ap(), in_=t)
            nc.sync.dma_start(
                out=dbg["dVP"].ap(),
                in_=vprime.rearrange("p a b c -> p (a b c)"))

    nc.finalize()
    return nc


def shard_inputs(x, Wq, bq, Wk, bk, Wv, bv, Wo, bo, n_cores=N_CORES):
    """Full inputs -> per-core input maps (host-side transpose/slice/reshape)."""
    H = Wq.shape[1]
    cores_per_batch = n_cores // x.shape[0]
    hl = H // cores_per_batch
    in_maps = []
    for c in range(n_cores):
        b = c // cores_per_batch
        h0 = (c % cores_per_batch) * hl
        bf = ml_dtypes.bfloat16
        m = {
            "xT": np.ascontiguousarray(x[b].T).astype(bf),
            "wq": np.ascontiguousarray(Wq[:, h0:h0 + hl, :].reshape(Wq.shape[0], -1)).astype(bf),
            "wk": np.ascontiguousarray(Wk[:, h0:h0 + hl, :].reshape(Wk.shape[0], -1)).astype(bf),
            "wv": np.ascontiguousarray(Wv[:, h0:h0 + hl, :].reshape(Wv.shape[0], -1)).astype(bf),
            "wo": np.ascontiguousarray(Wo[h0:h0 + hl].reshape(-1, Wo.shape[2])).astype(bf),
        }
        if _use_bias(bq, bk, bv):
            m["bq"] = np.ascontiguousarray(bq[h0:h0 + hl].reshape(1, -1)).astype(np.float32)
            m["bk"] = np.ascontiguousarray(bk[h0:h0 + hl].reshape(1, -1)).astype(np.float32)
            m["bv"] = np.ascontiguousarray(bv[h0:h0 + hl].reshape(1, -1)).astype(np.float32)
        in_maps.append(m)
    return in_maps


def _use_bias(bq, bk, bv):
    return bool(np.any(bq) or np.any(bk) or np.any(bv))


_ACT_ROOT_READY = False


def _ensure_act_root():
    """Point walrus at an act-table root whose only set is
    natural_log_exp_and_others, so exp and ln share one ACT table set and the
    kernel never pays mid-stream ACT_TABLE_LOADs."""
    global _ACT_ROOT_READY
    if _ACT_ROOT_READY or os.environ.get("BASS_ACT_ROOT_JSON_PATH"):
        _ACT_ROOT_READY = True
        return
    import json
    import tempfile
    from neuronxcc.driver.Job import Job
    from neuronxcc.driver.jobs.support.FindActInfo import findActInfoFile

    orig = findActInfoFile(Job.getPackageDir(), "gen3")
    with open(orig) as f:
        info = json.load(f)
    keep = [e for e in info["act_func_sets"]
            if e["name"] == "natural_log_exp_and_others"]
    if not keep:
        _ACT_ROOT_READY = True
        return
    root = tempfile.mkdtemp(prefix="act_root_")
    src_dir = os.path.dirname(orig)
    for fn in os.listdir(src_dir):
        if fn != "act_info.json":
            os.symlink(os.path.join(src_dir, fn), os.path.join(root, fn))
    info["act_func_sets"] = keep
    with open(os.path.join(root, "act_info.json"), "w") as f:
        json.dump(info, f)
    os.environ["BASS_ACT_ROOT_JSON_PATH"] = os.path.join(root, "act_info.json")

    import concourse.hw_specs as hw_specs
    import concourse.bacc as bacc_mod
    _orig_tables = hw_specs.get_activation_tables

    def _single_set_tables(module_arch):
        tables = _orig_tables(module_arch)
        if "natural_log_exp_and_others" in tables:
            return {"natural_log_exp_and_others": tables["natural_log_exp_and_others"]}
        return tables

    hw_specs.get_activation_tables = _single_set_tables
    bacc_mod.get_activation_tables = _single_set_tables
    _ACT_ROOT_READY = True


_NC_CACHE = {}


def _get_nc(use_bias):
    if use_bias not in _NC_CACHE:
        _NC_CACHE[use_bias] = build_nc(use_bias=use_bias)
    return _NC_CACHE[use_bias]


def run_sharded(inputs, trace=False, trace_cores=None):
    """Run the SPMD kernel; returns (full_output, BassKernelResults)."""
    x, bo = inputs["x"], inputs["bo"]
    use_bias = _use_bias(inputs["bq"], inputs["bk"], inputs["bv"])
    _ensure_act_root()
    nc = _get_nc(use_bias)
    in_maps = shard_inputs(**inputs)
    res = bass_utils.run_bass_kernel_spmd(
        nc, in_maps, core_ids=list(range(N_CORES)),
        trace=trace, trace_cores=trace_cores)
    cores_per_batch = N_CORES // x.shape[0]
    out = np.empty_like(x)
    for b in range(x.shape[0]):
        acc = np.zeros((x.shape[2], x.shape[1]), np.float32)
        for c in range(b * cores_per_batch, (b + 1) * cores_per_batch):
            acc += res.results[c]["outT"].astype(np.float32)
        out[b] = acc.T + bo[None, :]
    return out, res


def kernel(**inputs):
    out, _ = run_sharded(inputs)
    return out
